# revision 42
# baseline (speedup 1.0000x reference)
"""Trainium2 Bass kernel for quantized InvertedResidual block (DoReFa fake-quant).

Strategy (v5):
- Data-parallel: 32 images -> 4 per core across 8 NeuronCores.
- All matmuls fp16 (1 cycle/row on PE vs 4 for fp32):
  stage1: x split into fp16 hi+lo (22 mantissa bits), packed as K=128
          with duplicated integer weights [w1i; w1i] (w1i = w1q*255,
          exact in fp16) -> one matmul per (group, pixel tile).
  stage2: depthwise 3x3: PE groups = 9 diagonal matmuls; DVE group(s)
          = per-partition FMA chain (scalar_tensor_tensor) over wide
          (4/3-tile) units with fp32 SBUF accumulation.
  stage3: 1x1 conv with integer weights w3q*255; two pixel tiles are
          stacked into the 128 partitions (64+64) so drains/clamps
          run at full width.
- fp16 magic rounding: ACT computes scale*psum + (bias + 1024) in fp32;
  its fp16 output downcast rounds to the integer grid via RNE.
  Activations are stored offset by +1024; the next stage's bias absorbs
  1024*sum(weights) exactly. One DVE clamp (max 1024, min 1279) per
  tile finishes the quant.
- residual on host: the kernel outputs the clamped stage-3 integer grid
  u3 (fp16); the host computes y = x + (u3 - 1024)/255. This removes
  the xp input stream, the fp32 output stream, and the DVE residual op.
- Software pipeline: image i+1's stage-1 units are interleaved into
  image i's stage-2 stream, and image i's stage-3 pairs are deferred
  into image i+1's stream (h1/h2 are double-buffered by image parity),
  so the PE never stalls on the slow DVE depthwise chains. The last
  image keeps its late tiles on the PE and drains its stage-3 pairs
  as soon as their h2 completes.
- DMA plan: per-DMA completion costs ~2.5us serialized per HWDGE ring,
  so the prologue orders DMAs by need time across the sync + scalar
  rings (xhl in 3 pipelined chunks, wdw split by group, all small
  consts packed into one tensor), and xhl is prefetched 2 images deep.
  Never use the gpsimd SWDGE ring (it slows the whole device ~20%).
"""
import numpy as np

EPS = 1e-5
OFF = np.float32(1024.0)   # fp16 magic offset: [1024, 2048) has ulp 1

B, C, H, W = 32, 64, 56, 56
HID = 384
NCORES = 8
BPC = B // NCORES          # images per core
PIX = H * W                # 3136
PW = W + 2                 # 58
PH = H + 2
PPIX = PW * PH             # 3364
NT = 7                     # pixel tiles per image
TW = PIX // NT             # 448 = 8 rows x 56
ROWS_PT = H // NT          # 8
NG = HID // 128            # 3 channel groups
NU = (NT + 1) // 2         # 4 double-width units per (group, image)
NPAIR = (NT + 1) // 2      # 4 stage-3 tile pairs (last is a solo)
OW = NPAIR * TW            # 1792 output cols (stacked pairs)

# stage-2 assignment: per-image list of (g, t0, nt) tile ranges handled on
# DVE (wide scalar_tensor_tensor units); everything else runs on the PE as
# diagonal matmuls. The last image keeps its late tiles on the PE so the
# stage-3 tail is not gated on the (slow) DVE chain.
DVE_ASSIGN = [
    [(2, 0, 4), (2, 4, 3)],
    [(2, 0, 4), (2, 4, 3)],
    [(2, 0, 4), (2, 4, 3)],
    [(2, 0, 4)],
]
# tile ranges on the DVE-multiply + GpSimd-accumulate lane (per image)
GPS_ASSIGN = [
    [],
    [],
    [],
    [],
]

_cache = {}


def _quant_w(w):
    # DoReFa weight fake-quant, computed with jax on CPU so tanh/round are
    # bitwise identical to the reference implementation.
    import jax
    import jax.numpy as jnp
    with jax.default_device(jax.devices('cpu')[0]):
        t = jnp.tanh(jnp.asarray(w, jnp.float32))
        m = jnp.max(jnp.abs(t), axis=(1, 2, 3), keepdims=True)
        wn = t / (2.0 * m) + 0.5
        q = 2.0 * jnp.round(wn * 255.0) / 255.0 - 1.0
        return np.asarray(q, np.float32)


def _build_program():
    import concourse.bass as bass
    import concourse.tile as tile
    from concourse import bacc, mybir

    fp32 = mybir.dt.float32
    f16 = mybir.dt.float16
    nc = bacc.Bacc("TRN2", target_bir_lowering=False, debug=False,
                   enable_asserts=False, num_devices=NCORES)

    xhl = nc.dram_tensor("xhl", [BPC, 128, PIX], f16, kind="ExternalInput").ap()
    w1s = nc.dram_tensor("w1s", [128, HID], f16, kind="ExternalInput").ap()
    wdw = nc.dram_tensor("wdw", [128, NG * 9 * 128], f16, kind="ExternalInput").ap()
    w3i = nc.dram_tensor("w3i", [128, NG * 64], f16, kind="ExternalInput").ap()
    sb1 = nc.dram_tensor("sb1", [128, 2 * NG], fp32, kind="ExternalInput").ap()
    # all remaining small fp32 consts packed into one tensor / one DMA:
    # [s2m(3) | b2m(3) | s3m(1) | b3m(1) | wdv(27) | b2g(3) | wdvo(27)]
    cst = nc.dram_tensor("cst", [128, 65], fp32, kind="ExternalInput").ap()
    ys = nc.dram_tensor("ys", [BPC, 128, OW], f16, kind="ExternalOutput").ap()

    mx = mybir.AluOpType.max
    mn = mybir.AluOpType.min
    add = mybir.AluOpType.add
    mult = mybir.AluOpType.mult
    IDENT = mybir.ActivationFunctionType.Identity

    CLO = float(OFF)           # 1024.0
    CHI = float(OFF) + 255.0   # 1279.0
    TAPS = [(dy, dx) for dy in (-1, 0, 1) for dx in (-1, 0, 1)]

    with tile.TileContext(nc) as tc:
        from contextlib import ExitStack
        with ExitStack() as ctx:
            consts = ctx.enter_context(tc.tile_pool(name="consts", bufs=1))
            h1p_pool = ctx.enter_context(tc.tile_pool(name="h1p", bufs=1))
            h2_pool = ctx.enter_context(tc.tile_pool(name="h2", bufs=1))
            x_pool = ctx.enter_context(tc.tile_pool(name="x", bufs=3))
            o_pool = ctx.enter_context(tc.tile_pool(name="o", bufs=3))
            v1_pool = ctx.enter_context(tc.tile_pool(name="v1", bufs=4))
            v2_pool = ctx.enter_context(tc.tile_pool(name="v2", bufs=4))
            vw_pool = ctx.enter_context(tc.tile_pool(name="vw", bufs=2))
            v3_pool = ctx.enter_context(tc.tile_pool(name="v3", bufs=5))
            accw_pool = ctx.enter_context(tc.tile_pool(name="accw", bufs=2))
            accd_pool = ctx.enter_context(tc.tile_pool(name="accd", bufs=3))
            gtmp_pool = ctx.enter_context(tc.tile_pool(name="gtmp", bufs=3))
            accg_pool = ctx.enter_context(tc.tile_pool(name="accg", bufs=1))
            # PSUM: pa 1x2 banks + pb 2x2 banks + pc 2x1 bank = 8 banks
            pa_pool = ctx.enter_context(tc.tile_pool(name="pa", bufs=1, space="PSUM"))
            pb_pool = ctx.enter_context(tc.tile_pool(name="pb", bufs=2, space="PSUM"))
            pc_pool = ctx.enter_context(tc.tile_pool(name="pc", bufs=2, space="PSUM"))

            # scalar ring: sb1 (tiny, unblocks prologue drains) then w1s
            # (gates the first MM's weight load). The xhl head goes first on
            # the sync ring in parallel — per-DMA completion costs ~2.5us
            # serialized per ring, so order = need-time order.
            w1s_sb = consts.tile([128, HID], f16)
            nc.scalar.dma_start(w1s_sb[:], w1s)
            sb1_sb = consts.tile([128, 2 * NG], fp32)
            nc.scalar.dma_start(sb1_sb[:], sb1)

            def late_consts():
                # wdw split by group: g0's columns complete first, matching
                # the first PE stage-2 unit's need time.
                wdw_sb = consts.tile([128, NG * 9 * 128], f16)
                nc.sync.dma_start(wdw_sb[:, 0:9 * 128], wdw[:, 0:9 * 128])
                nc.sync.dma_start(wdw_sb[:, 9 * 128:], wdw[:, 9 * 128:])
                cst_sb = consts.tile([128, 65], fp32)
                nc.scalar.dma_start(cst_sb[:], cst)
                w3i_sb = consts.tile([128, NG * 64], f16)
                nc.scalar.dma_start(w3i_sb[:], w3i)
                s2m_sb = cst_sb[:, 0:3]
                b2m_sb = cst_sb[:, 3:6]
                s3m_sb = cst_sb[:, 6:7]
                b3m_sb = cst_sb[:, 7:8]
                wdv_sb = cst_sb[:, 8:35]
                b2g_sb = cst_sb[:, 35:38]
                wdvo_sb = cst_sb[:, 38:65]
                return (wdw_sb, w3i_sb, s2m_sb, b2m_sb, s3m_sb, b3m_sb,
                        wdv_sb, b2g_sb, wdvo_sb)

            # persistent padded H1 (offset integer grid r1+1024), double
            # buffered by image parity; borders hold 1024 (= r1 of 0) so
            # the absorbed-offset bias correction is exact at edges too.
            h1p = [[h1p_pool.tile([128, PPIX], f16, tag=f"h1p{p}{g}",
                                  name=f"h1p{p}{g}") for g in range(NG)]
                   for p in range(2)]
            h1v = [[t[:].rearrange("p (h w) -> p h w", w=PW) for t in h1p[p]]
                   for p in range(2)]
            # only the 1-pixel border needs the 1024 fill (the interior is
            # overwritten every image); whole-tile memsets would serialize
            # ~17us on GpSimd before stage-1 can write.
            for p in range(2):
                for g in range(NG):
                    hv = h1v[p][g]
                    nc.gpsimd.memset(hv[:, 0:1, :], float(OFF))
                    nc.gpsimd.memset(hv[:, PH - 1:PH, :], float(OFF))
                    nc.gpsimd.memset(hv[:, 1:PH - 1, 0:1], float(OFF))
                    nc.gpsimd.memset(hv[:, 1:PH - 1, PW - 1:PW], float(OFF))
            h2t = [[h2_pool.tile([128, PIX], f16, tag=f"h2{p}{g}",
                                 name=f"h2{p}{g}") for g in range(NG)]
                   for p in range(2)]

            def dma_in(i, split=False):
                xhl_sb = x_pool.tile([128, PIX], f16, tag="xhl")
                if split:
                    # three pipelined chunks across both rings, arriving in
                    # stage-1 unit order (tiles 0-1, 2-3, 4-6).
                    nc.sync.dma_start(xhl_sb[:, 0:2 * TW], xhl[i, :, 0:2 * TW])
                    nc.scalar.dma_start(xhl_sb[:, 2 * TW:4 * TW],
                                        xhl[i, :, 2 * TW:4 * TW])
                    nc.sync.dma_start(xhl_sb[:, 4 * TW:PIX],
                                      xhl[i, :, 4 * TW:PIX])
                else:
                    nc.sync.dma_start(xhl_sb[:], xhl[i, :, :])
                return xhl_sb

            def emit_s1(i, g, u, xhl_sb, pool=None):
                p = i % 2
                nt = 2 if u < NU - 1 else NT - 2 * (NU - 1)
                w = TW * nt
                pa = (pa_pool if pool is None else pool).tile(
                    [128, 1024], fp32,
                    tag="pa" if pool is None else "pb", name="pa")
                for j in range(nt):
                    t = 2 * u + j
                    nc.tensor.matmul(
                        pa[:, 512 * j:512 * j + TW],
                        w1s_sb[:, 128 * g:128 * (g + 1)],
                        xhl_sb[:, TW * t:TW * (t + 1)],
                        start=True, stop=True)
                pav = pa[:].rearrange("q (b c) -> q b c", c=512)[:, 0:nt, 0:TW]
                v = v1_pool.tile([128, 2 * TW], f16)
                nc.scalar.activation(v[:, 0:w], pav, IDENT,
                                     bias=sb1_sb[:, NG + g:NG + g + 1],
                                     scale=sb1_sb[:, g:g + 1])
                r0 = ROWS_PT * 2 * u + 1
                nc.vector.tensor_scalar(
                    h1v[p][g][:, r0:r0 + ROWS_PT * nt, 1:57], v[:, 0:w],
                    CLO, CHI, op0=mx, op1=mn)

            def emit_s2(i, g, u):
                p = i % 2
                nt = 2 if u < NU - 1 else NT - 2 * (NU - 1)
                w = TW * nt
                pb = pb_pool.tile([128, 1024], fp32)
                # tap-major: consecutive matmuls share lhsT
                for k, (dy, dx) in enumerate(TAPS):
                    lcol = 128 * (9 * g + k)
                    for j in range(nt):
                        t = 2 * u + j
                        r0 = ROWS_PT * t + 1
                        rhs = h1v[p][g][:, r0 + dy:r0 + dy + ROWS_PT,
                                        1 + dx:57 + dx]
                        nc.tensor.matmul(
                            pb[:, 512 * j:512 * j + TW],
                            wdw_sb[:, lcol:lcol + 128], rhs,
                            start=(k == 0), stop=(k == 8))
                pbv = pb[:].rearrange("q (b c) -> q b c", c=512)[:, 0:nt, 0:TW]
                v = v2_pool.tile([128, 2 * TW], f16)
                nc.scalar.activation(v[:, 0:w], pbv, IDENT,
                                     bias=b2m_sb[:, g:g + 1],
                                     scale=s2m_sb[:, g:g + 1])
                nc.vector.tensor_scalar(
                    h2t[p][g][:, 2 * TW * u:2 * TW * u + w], v[:, 0:w],
                    CLO, CHI, op0=mx, op1=mn)

            def emit_s2_vec(i, g, t0, nt, acc_pool, vpool, vtag):
                # wide depthwise unit on DVE: 9 per-partition FMAs with
                # fp32 SBUF accumulation (exact: integer values), then the
                # usual ACT round + clamp.
                p = i % 2
                w = TW * nt
                rows = ROWS_PT * nt
                r0 = ROWS_PT * t0 + 1
                cur = acc_pool.tile([128, w], fp32, tag="a", name="acc_a")
                nxt = acc_pool.tile([128, w], fp32, tag="b", name="acc_b")
                for k, (dy, dx) in enumerate(TAPS):
                    win = h1v[p][g][:, r0 + dy:r0 + dy + rows, 1 + dx:57 + dx]
                    wap = wdv_sb[:, 9 * g + k:9 * g + k + 1]
                    if k == 0:
                        # first tap on ACT: w[c]*win with per-partition scale
                        nc.scalar.mul(cur[:, 0:w], win, wap)
                    else:
                        nc.vector.scalar_tensor_tensor(nxt[:, 0:w], win, wap,
                                                       cur[:, 0:w],
                                                       op0=mult, op1=add)
                        cur, nxt = nxt, cur
                v = vpool.tile([128, w], f16, tag=vtag)
                nc.scalar.activation(v[:, 0:w], cur[:, 0:w], IDENT,
                                     bias=b2m_sb[:, g:g + 1],
                                     scale=s2m_sb[:, g:g + 1])
                nc.vector.tensor_scalar(
                    h2t[p][g][:, TW * t0:TW * t0 + w], v[:, 0:w],
                    CLO, CHI, op0=mx, op1=mn)

            def emit_s2_gps(i, g, t0, nt):
                # depthwise unit on the DVE-multiply + GpSimd-accumulate
                # lane: DVE computes tmp_k = w_k*win - 1024*w_k (fp16, fast
                # 1-input mode); GpSimd chains acc += tmp_k in fp32. The
                # per-tap -1024*w_k removes the h1 offset so tmp fits fp16.
                p = i % 2
                w = TW * nt
                rows = ROWS_PT * nt
                r0 = ROWS_PT * t0 + 1
                acc = [accg_pool.tile([128, w], fp32, tag="ga", name="acc_ga"),
                       accg_pool.tile([128, w], fp32, tag="gb", name="acc_gb")]
                for k, (dy, dx) in enumerate(TAPS):
                    win = h1v[p][g][:, r0 + dy:r0 + dy + rows, 1 + dx:57 + dx]
                    wap = wdv_sb[:, 9 * g + k:9 * g + k + 1]
                    wop = wdvo_sb[:, 9 * g + k:9 * g + k + 1]
                    if k == 0:
                        nc.vector.tensor_scalar(
                            acc[0][:, 0:w].rearrange("q (r c) -> q r c", c=56),
                            win, wap, wop, op0=mult, op1=add)
                    else:
                        tmp = gtmp_pool.tile([128, w], f16, tag="t",
                                             name="gtmp")
                        nc.vector.tensor_scalar(
                            tmp[:, 0:w].rearrange("q (r c) -> q r c", c=56),
                            win, wap, wop, op0=mult, op1=add)
                        nc.gpsimd.tensor_tensor(
                            acc[k % 2][:, 0:w], tmp[:, 0:w],
                            acc[(k + 1) % 2][:, 0:w], add)
                v = vw_pool.tile([128, w], f16, tag="vg")
                nc.scalar.activation(v[:, 0:w], acc[8 % 2][:, 0:w], IDENT,
                                     bias=b2g_sb[:, g:g + 1],
                                     scale=s2m_sb[:, g:g + 1])
                nc.vector.tensor_scalar(
                    h2t[p][g][:, TW * t0:TW * t0 + w], v[:, 0:w],
                    CLO, CHI, op0=mx, op1=mn)

            def emit_s3(i, j, o_sb):
                # stage-3 tile pair (2j, 2j+1) stacked into 128 partitions
                p = i % 2
                tiles = [2 * j] if 2 * j + 1 >= NT else [2 * j, 2 * j + 1]
                npart = 64 * len(tiles)
                pc = pc_pool.tile([128, TW], fp32)
                for h, t in enumerate(tiles):
                    for kc in range(NG):
                        nc.tensor.matmul(
                            pc[64 * h:64 * h + 64, :],
                            w3i_sb[:, 64 * kc:64 * (kc + 1)],
                            h2t[p][kc][:, TW * t:TW * (t + 1)],
                            start=(kc == 0), stop=(kc == NG - 1))
                v3 = v3_pool.tile([128, TW], f16, tag="v3", name="v3")
                nc.scalar.activation(v3[0:npart, :], pc[0:npart, :], IDENT,
                                     bias=b3m_sb[0:npart, 0:1],
                                     scale=s3m_sb[0:npart, 0:1])
                nc.vector.tensor_scalar(
                    o_sb[0:npart, TW * j:TW * (j + 1)], v3[0:npart, :],
                    CLO, CHI, op0=mx, op1=mn)

            # stage-1 unit order: DVE-assigned groups first so their h1 is
            # ready as early as possible for the slow DVE chains.
            S1_UNITS = [(g, u) for g in (2, 0, 1) for u in range(NU)]

            def s2_unit_list(i):
                # stage-2 unit descriptors:
                #   ('pe', g, u) | ('dve', g, t0, nt) | ('gps', g, t0, nt)
                dve = DVE_ASSIGN[i]
                gps = GPS_ASSIGN[i]
                dve_tiles = {(g, t) for (g, t0, nt) in dve
                             for t in range(t0, t0 + nt)}
                dve_tiles |= {(g, t) for (g, t0, nt) in gps
                              for t in range(t0, t0 + nt)}
                units = []
                gorder = (2, 0, 1) if i == BPC - 1 else (0, 1, 2)
                for g in gorder:
                    for u in range(NU):
                        ts = range(2 * u, min(2 * u + 2, NT))
                        if any((g, t) in dve_tiles for t in ts):
                            assert all((g, t) in dve_tiles for t in ts)
                        else:
                            units.append(('pe', g, u))
                wide = ([('gps', g, t0, nt) for (g, t0, nt) in gps]
                        + [('dve', g, t0, nt) for (g, t0, nt) in dve])
                # interleave: start the wide DVE units early (long latency)
                out = []
                for k in range(max(len(units), len(wide))):
                    if k < len(wide):
                        out.append(wide[k])
                    if k < len(units):
                        out.append(units[k])
                return out

            def do_s3(i, j, o_sb, ring):
                emit_s3(i, j, o_sb)
                # stream output while the rest computes. The very last write
                # (pair 3 of the last image, 64 partitions) splits into two
                # column halves across both rings so the completion sems that
                # gate the NEFF end barrier fire in parallel.
                if i == BPC - 1 and j == NPAIR - 1:
                    h = TW // 2
                    nc.sync.dma_start(ys[i, 0:64, TW * j:TW * j + h],
                                      o_sb[0:64, TW * j:TW * j + h])
                    nc.scalar.dma_start(ys[i, 0:64, TW * j + h:TW * (j + 1)],
                                        o_sb[0:64, TW * j + h:TW * (j + 1)])
                    return
                eng = nc.sync if ring % 2 == 0 else nc.scalar
                eng.dma_start(ys[i, :, TW * j:TW * (j + 1)],
                              o_sb[:, TW * j:TW * (j + 1)])

            bufs = {0: dma_in(0, split=True)}
            (wdw_sb, w3i_sb, s2m_sb, b2m_sb, s3m_sb, b3m_sb,
             wdv_sb, b2g_sb, wdvo_sb) = late_consts()
            bufs[1] = dma_in(1)
            # image-0 stage-1 prologue: alternate the pa/pb PSUM pools (pb is
            # idle before the first stage-2 unit) so ACT drains double-buffer
            # instead of serializing on the single pa buffer.
            for k, (g, u) in enumerate(S1_UNITS):
                emit_s1(0, g, u, bufs[0],
                        pool=None if k % 2 == 0 else pb_pool)
            prev = None        # (img, o_sb, pending s3 pair list)
            for i in range(BPC):
                last = i + 1 >= BPC
                if i + 2 < BPC:
                    bufs[i + 2] = dma_in(i + 2)
                o_sb = o_pool.tile([128, OW], f16)
                s3q = list(range(NPAIR))
                units = s2_unit_list(i)
                # tiles of each group completed so far this image
                done = {g: 0 for g in range(NG)}

                def maybe_s3(i, o_sb):
                    # drain stage-3 pairs whose h2 inputs are complete
                    for j in list(s3q):
                        need = min(2 * j + 2, NT)
                        if all(done[g] >= need for g in range(NG)):
                            s3q.remove(j)
                            do_s3(i, j, o_sb, j)

                s1_iter = iter(S1_UNITS)
                nunits = len(units)
                for idx, desc in enumerate(units):
                    if desc[0] == 'pe':
                        _, g, u = desc
                        emit_s2(i, g, u)
                        done[g] += 2 if u < NU - 1 else NT - 2 * (NU - 1)
                    elif desc[0] == 'gps':
                        _, g, t0, nt = desc
                        emit_s2_gps(i, g, t0, nt)
                        done[g] += nt
                    else:
                        _, g, t0, nt = desc
                        if nt > 2:
                            emit_s2_vec(i, g, t0, nt, accw_pool, vw_pool, "vw")
                        else:
                            emit_s2_vec(i, g, t0, nt, accd_pool, v2_pool,
                                        "v2d")
                        done[g] += nt
                    if not last:
                        nu = next(s1_iter, None)
                        if nu is not None:
                            emit_s1(i + 1, nu[0], nu[1], bufs[i + 1])
                    # pace the previous image's stage-3 pairs evenly through
                    # this image's stage-2 stream (their h2 is complete).
                    if prev is not None and prev[2]:
                        want = (idx + 1) * NPAIR // nunits
                        while len(prev[2]) > NPAIR - want:
                            j = prev[2].pop(0)
                            do_s3(prev[0], j, prev[1], j)
                    if last:
                        maybe_s3(i, o_sb)
                if not last:
                    for nu in s1_iter:
                        emit_s1(i + 1, nu[0], nu[1], bufs[i + 1])
                if prev is not None:
                    for j in prev[2]:
                        do_s3(prev[0], j, prev[1], j)
                if last:
                    for j in list(s3q):
                        s3q.remove(j)
                        do_s3(i, j, o_sb, j)
                prev = (i, o_sb, s3q if not last else [])
                del bufs[i]

    nc.compile()
    return nc


def _prep_weights(inputs):
    inv1 = (inputs['g1'] / np.sqrt(inputs['v1'] + EPS)).astype(np.float32)
    beta1 = (inputs['b1'] - inputs['m1'] * inv1).astype(np.float32)
    inv2 = (inputs['g2'] / np.sqrt(inputs['v2'] + EPS)).astype(np.float32)
    beta2 = (inputs['b2'] - inputs['m2'] * inv2).astype(np.float32)
    inv3 = (inputs['g3'] / np.sqrt(inputs['v3'] + EPS)).astype(np.float32)
    beta3 = (inputs['b3'] - inputs['m3'] * inv3).astype(np.float32)

    w1q = _quant_w(inputs['w1'])[:, :, 0, 0]       # [384, 64]
    w2q = _quant_w(inputs['w2'])[:, 0, :, :]       # [384, 3, 3]
    w3q = _quant_w(inputs['w3'])[:, :, 0, 0]       # [64, 384]

    # integer weights (w*255 is an exact odd integer <= 255, fp16-exact)
    w1i = np.round(w1q * 255.0).astype(np.float32).T        # [64, 384]
    w1s = np.concatenate([w1i, w1i], axis=0).astype(np.float16)  # [128, 384]

    wdw_int = np.round(w2q * 255.0).astype(np.float32)      # [384, 3, 3]
    wdw = np.zeros((128, NG * 9 * 128), np.float16)
    for g in range(NG):
        ch = slice(128 * g, 128 * (g + 1))
        k = 0
        for dy in range(3):
            for dx in range(3):
                col = 128 * (9 * g + k)
                wdw[:, col:col + 128][np.arange(128), np.arange(128)] = \
                    wdw_int[ch, dy, dx].astype(np.float16)
                k += 1

    w3int = np.round(w3q * 255.0).astype(np.float32)        # [64, 384]
    w3i = np.zeros((128, NG * 64), np.float16)
    for kc in range(NG):
        w3i[:, 64 * kc:64 * (kc + 1)] = \
            w3int[:, 128 * kc:128 * (kc + 1)].T.astype(np.float16)

    # stage-1: t1 = pa*(inv1/6) + beta1*42.5 ; +1024 fp16 magic
    s1 = (inv1 / np.float32(6.0)).astype(np.float32)
    b1 = (beta1 * np.float32(42.5) + OFF).astype(np.float32)
    s1m = s1.reshape(NG, 128).T.copy()
    b1m = b1.reshape(NG, 128).T.copy()

    # stage-2: inputs carry +1024; absorb 1024*sum(w2) into the bias
    sumw2 = wdw_int.sum(axis=(1, 2)).astype(np.float32)     # [384]
    s2 = (inv2 / np.float32(255.0)).astype(np.float32)
    b2 = (beta2 * np.float32(42.5) + OFF - s2 * OFF * sumw2).astype(np.float32)
    s2m = s2.reshape(NG, 128).T.copy()
    b2m = b2.reshape(NG, 128).T.copy()

    # stage-3: inputs carry +1024; absorb 1024*sum(w3) into the bias;
    # params duplicated across both 64-partition halves (tile stacking)
    sumw3 = w3int.sum(axis=1).astype(np.float32)            # [64]
    s3 = (np.float32(6.0) * inv3 / np.float32(255.0)).astype(np.float32)
    b3 = (beta3 * np.float32(255.0) + OFF - s3 * OFF * sumw3).astype(np.float32)
    s3m = np.concatenate([s3, s3]).reshape(128, 1)
    b3m = np.concatenate([b3, b3]).reshape(128, 1)

    # per-partition tap weights for vector-engine depthwise units
    wdv = np.zeros((128, NG * 9), np.float32)
    for g in range(NG):
        k = 0
        for dy in range(3):
            for dx in range(3):
                wdv[:, 9 * g + k] = wdw_int[128 * g:128 * (g + 1), dy, dx]
                k += 1

    sb1 = np.concatenate([s1m, b1m], axis=1)
    # gps-lane consts: bias without the offset-sum correction (the lane's
    # accumulator is already offset-free), and the per-tap -1024*w terms
    b2g = (beta2 * np.float32(42.5) + OFF).astype(np.float32)
    b2gm = b2g.reshape(NG, 128).T.copy()
    wdvo = (-np.float32(OFF)) * wdv
    cst = np.concatenate([s2m, b2m, s3m, b3m, wdv, b2gm, wdvo],
                         axis=1)  # [128, 65]
    return (w1s, np.ascontiguousarray(wdw), np.ascontiguousarray(w3i),
            np.ascontiguousarray(sb1), np.ascontiguousarray(cst))


def _make_inmaps(inputs):
    (w1s, wdw, w3i, sb1, cst) = _prep_weights(inputs)
    x = np.asarray(inputs['x'], np.float32).reshape(B, C, PIX)
    x_hi = x.astype(np.float16)
    x_lo = (x - x_hi.astype(np.float32)).astype(np.float16)
    xhl = np.concatenate([x_hi, x_lo], axis=1)              # [B, 128, PIX]

    in_maps = []
    for c in range(NCORES):
        sl = slice(BPC * c, BPC * (c + 1))
        in_maps.append({'xhl': np.ascontiguousarray(xhl[sl]),
                        'w1s': w1s, 'wdw': wdw, 'w3i': w3i,
                        'sb1': sb1, 'cst': cst})
    return in_maps


def _unpack_out(inputs, results):
    # ys[i, 0:64, 448j:448j+448] = tile 2j ; ys[i, 64:128, ...] = tile 2j+1
    x = np.asarray(inputs['x'], np.float32).reshape(B, C, PIX)
    u3 = np.empty((B, C, PIX), np.float32)
    for c in range(NCORES):
        ysc = results[c]['ys'].astype(np.float32)           # [BPC, 128, OW]
        for j in range(NPAIR):
            t0 = 2 * j
            u3[BPC * c:BPC * (c + 1), :, TW * t0:TW * (t0 + 1)] = \
                ysc[:, 0:64, TW * j:TW * (j + 1)]
            if t0 + 1 < NT:
                u3[BPC * c:BPC * (c + 1), :, TW * (t0 + 1):TW * (t0 + 2)] = \
                    ysc[:, 64:128, TW * j:TW * (j + 1)]
    y = x + (u3 - np.float32(OFF)) * np.float32(1.0 / 255.0)
    return y.reshape(B, C, H, W).astype(np.float32)


def kernel(**inputs):
    from concourse import bass_utils

    if 'nc' not in _cache:
        _cache['nc'] = _build_program()
    nc = _cache['nc']

    in_maps = _make_inmaps(inputs)
    res = bass_utils.run_bass_kernel_spmd(nc, in_maps, list(range(NCORES)))
    return _unpack_out(inputs, res.results)
